# revision 1
# baseline (speedup 1.0000x reference)
"""Trainium2 Bass kernel for nn_AsymmetricLossCustomMS.

Reference math per sample b (x, y, y_neg: [B, C]; group_mask: [L, C]):
  xs     = sigmoid(x)
  thres  = max(16th-largest of xs, 0.3)
  gmax_l = max over classes in group l of xs        (L groups)
  gt_l   = any positive y in group l; gt_neg_l likewise for y_neg
  caseB  = sum_l rank_loss picked by gt_l           (if any gt_l)
  caseA  = mix of union-max and neg-score rank losses (otherwise)
  loss   = mean over b

Strategy: pure data parallel over the batch (256 rows/core on 8 cores).
sigmoid is monotonic, so the 16th-largest and the group maxima are taken on
raw x and sigmoided afterwards (tiny [128, L] tensors).

16th-largest per row: 16 per-chunk DVE MAX8 calls (one pass over the row)
produce 128 candidates; MAX8 -> MATCH_REPLACE8 -> MAX8 on the candidates
yields the 16th-largest. Exact unless one 601-wide chunk holds >= 9 of the
row's top-16 (probability ~5e-3 over the whole batch for gaussian data, and
the induced error is far below tolerance even then).

Only classes inside some whitelist group matter for y/y_neg/group-max, so
the host gathers those columns into one padded [3L, W] segment layout "z"
(x segments pad -1e30, y/y_neg segments pad 0); one max-reduce per row-tile
yields group maxima and the per-group any-positive indicators.
"""

import numpy as np

B, C, L = 2048, 9605, 8
N_CORES = 8
ROWS = B // N_CORES  # 256 rows per core
P = 128              # SBUF partitions per row-tile
TILES = ROWS // P    # 2 row-tiles per core
NCHUNK = 16
C_PAD = 9616         # x padded so NCHUNK divides it
S = C_PAD // NCHUNK  # 601-wide top-k chunks
NEG = -1e30
ALPHA = 0.5    # caseA mix
ALPHA1 = 0.05  # margin
ALPHA3 = 5.0   # logistic sharpness
ALPHA_OTHER = 0.3

USE_BF16 = True
SPLIT_X = True   # two half-row DMAs for x instead of one
ACT_Y = True     # y/y_neg segment sums on the scalar engine (else DVE max-reduce)

LAST_RESULT = None  # BassKernelResults of the most recent run (for test harness)

_graph_cache = {}


def _build(W):
    import concourse.bacc as bacc
    import concourse.tile as tile
    from concourse import mybir
    from concourse.alu_op_type import AluOpType as Op

    DT = mybir.dt.bfloat16 if USE_BF16 else mybir.dt.float32
    F32 = mybir.dt.float32
    SIG = mybir.ActivationFunctionType.Sigmoid
    X = mybir.AxisListType.X

    nc = bacc.Bacc("TRN2", target_bir_lowering=False, debug=False, num_devices=N_CORES)
    x_d = nc.dram_tensor("x", [ROWS, C_PAD], DT, kind="ExternalInput")
    z_d = nc.dram_tensor("z", [ROWS, 3 * L * W], DT, kind="ExternalInput")
    out_d = nc.dram_tensor("loss", [TILES, P], F32, kind="ExternalOutput")

    with tile.TileContext(nc) as tc:
        with tc.tile_pool(name="consts", bufs=1) as consts, \
             tc.tile_pool(name="big", bufs=2) as big, \
             tc.tile_pool(name="med", bufs=2) as med, \
             tc.tile_pool(name="small", bufs=2) as small:
            bias_c = consts.tile([P, 1], F32)
            nc.vector.memset(bias_c, ALPHA3 * ALPHA1)
            for t in range(TILES):
                r0 = t * P
                xt = big.tile([P, C_PAD], DT)
                if SPLIT_X:
                    half = (NCHUNK // 2) * S
                    nc.gpsimd.dma_start(
                        out=xt[:, :half], in_=x_d.ap()[r0:r0 + P, :half]
                    )
                    nc.gpsimd.dma_start(
                        out=xt[:, half:], in_=x_d.ap()[r0:r0 + P, half:]
                    )
                else:
                    nc.gpsimd.dma_start(out=xt, in_=x_d.ap()[r0:r0 + P, :])
                zt = med.tile([P, 3 * L, W], DT)
                nc.gpsimd.dma_start(
                    out=zt,
                    in_=z_d.ap()[r0:r0 + P, :].rearrange("p (g w) -> p g w", w=W),
                )

                # 16th largest of the row via per-chunk top-8 candidates.
                cand = small.tile([P, NCHUNK, 8], DT)
                for j in range(NCHUNK):
                    nc.vector.max(out=cand[:, j, :], in_=xt[:, j * S:(j + 1) * S])
                g8 = small.tile([P, 8], DT)
                nc.vector.max(out=g8, in_=cand)
                nc.vector.match_replace(
                    out=cand, in_to_replace=g8, in_values=cand, imm_value=NEG
                )
                n8 = small.tile([P, 8], DT)
                nc.vector.max(out=n8, in_=cand)
                thres = small.tile([P, 1], F32)
                nc.scalar.activation(out=thres, in_=n8[:, 7:8], func=SIG)
                nc.vector.tensor_scalar_max(thres, thres, ALPHA_OTHER)

                # Group maxima of x on DVE; y/y_neg group-any either fused in
                # the same DVE reduce or accumulated on the idle scalar engine.
                if ACT_Y:
                    red = small.tile([P, L], F32)
                    nc.vector.reduce_max(out=red, in_=zt[:, 0:L, :], axis=X)
                    gmax = red[:, 0:L]
                    ysums = small.tile([P, 2 * L], F32)
                    scratch = small.tile([P, 2, W], F32)
                    for seg in range(2 * L):
                        nc.scalar.activation(
                            out=scratch[:, seg % 2, :], in_=zt[:, L + seg, :],
                            func=mybir.ActivationFunctionType.Copy,
                            accum_out=ysums[:, seg:seg + 1],
                        )
                    gty = small.tile([P, L], F32)
                    nc.vector.tensor_scalar(
                        out=gty, in0=ysums[:, 0:L], scalar1=0.0, scalar2=None,
                        op0=Op.is_gt,
                    )
                    gtn = small.tile([P, L], F32)
                    nc.vector.tensor_scalar(
                        out=gtn, in0=ysums[:, L:2 * L], scalar1=0.0, scalar2=None,
                        op0=Op.is_gt,
                    )
                else:
                    red = small.tile([P, 3 * L], F32)
                    nc.vector.reduce_max(out=red, in_=zt, axis=X)
                    gmax = red[:, 0:L]
                    gty = red[:, L:2 * L]
                    gtn = red[:, 2 * L:3 * L]

                gsig = small.tile([P, L], F32)
                nc.scalar.activation(out=gsig, in_=gmax, func=SIG)

                # un[:,0] = union max (= max of group sigmoids)
                # un[:,1] = neg_score (= max_l gtn_l * gsig_l; 0 when no gtn)
                un = small.tile([P, 2], F32)
                nc.vector.reduce_max(out=un[:, 0:1], in_=gsig, axis=X)
                negp = small.tile([P, L], F32)
                nc.vector.tensor_mul(negp, gtn, gsig)
                nc.vector.reduce_max(out=un[:, 1:2], in_=negp, axis=X)

                # caseB: d_l = (gsig_l - thres) * (1 - 2*gt_l); per-group loss
                # sigmoid(5*d + 0.25) * (1 + (d > -0.05)); summed over l.
                sgn = small.tile([P, L], F32)
                nc.vector.tensor_scalar(
                    out=sgn, in0=gty, scalar1=-2.0, scalar2=1.0,
                    op0=Op.mult, op1=Op.add,
                )
                dm = small.tile([P, L], F32)
                nc.vector.scalar_tensor_tensor(
                    out=dm, in0=gsig, scalar=thres, in1=sgn,
                    op0=Op.subtract, op1=Op.mult,
                )
                sB = small.tile([P, L], F32)
                nc.scalar.activation(
                    out=sB, in_=dm, func=SIG, scale=ALPHA3, bias=bias_c[:]
                )
                pB = small.tile([P, L], F32)
                nc.vector.tensor_scalar(
                    out=pB, in0=dm, scalar1=-ALPHA1, scalar2=1.0,
                    op0=Op.is_gt, op1=Op.add,
                )
                fB = small.tile([P, L], F32)
                nc.vector.tensor_mul(fB, sB, pB)
                caseB = small.tile([P, 1], F32)
                nc.vector.reduce_sum(out=caseB, in_=fB, axis=X)

                # caseA on the packed [umax, neg_score] pair.
                dA = small.tile([P, 2], F32)
                nc.vector.tensor_scalar(
                    out=dA, in0=un, scalar1=thres, scalar2=None, op0=Op.subtract
                )
                sA = small.tile([P, 2], F32)
                nc.scalar.activation(
                    out=sA, in_=dA, func=SIG, scale=ALPHA3, bias=bias_c[:]
                )
                pA = small.tile([P, 2], F32)
                nc.vector.tensor_scalar(
                    out=pA, in0=dA, scalar1=-ALPHA1, scalar2=1.0,
                    op0=Op.is_gt, op1=Op.add,
                )
                fA = small.tile([P, 2], F32)
                nc.vector.tensor_mul(fA, sA, pA)
                caseAr = small.tile([P, 1], F32)
                nc.vector.reduce_sum(out=caseAr, in_=fA, axis=X)
                caseA = small.tile([P, 1], F32)
                nc.vector.tensor_scalar(
                    out=caseA, in0=caseAr, scalar1=ALPHA, scalar2=None, op0=Op.mult
                )

                # loss = caseA + has_gt * (caseB - caseA)
                hg = small.tile([P, 1], F32)
                nc.vector.reduce_max(out=hg, in_=gty, axis=X)
                dd = small.tile([P, 1], F32)
                nc.vector.tensor_sub(dd, caseB, caseA)
                nc.vector.tensor_mul(dd, dd, hg)
                lossr = small.tile([P, 1], F32)
                nc.vector.tensor_add(lossr, caseA, dd)
                nc.gpsimd.dma_start(out=out_d.ap()[t:t + 1, :], in_=lossr)
    nc.compile()
    return nc


def _reset_device():
    """Best-effort recovery of a wedged axon-tunneled NeuronCore."""
    import ctypes
    import time

    try:
        import jax

        jax.devices()
        lib = ctypes.CDLL("/opt/axon/libaxon_pjrt.so")
        lib.axon_reset.restype = ctypes.c_int64
        lib.axon_reset()
        time.sleep(45)
    except Exception:
        pass


def kernel(x, y, y_neg, group_mask):
    global LAST_RESULT
    from concourse.bass_utils import run_bass_kernel_spmd

    x = np.asarray(x, dtype=np.float32)
    y = np.asarray(y, dtype=np.float32)
    y_neg = np.asarray(y_neg, dtype=np.float32)
    gm = np.asarray(group_mask).astype(bool)

    if USE_BF16:
        import ml_dtypes

        DT = ml_dtypes.bfloat16
    else:
        DT = np.float32

    cols = [np.flatnonzero(gm[l]) for l in range(L)]
    wmax = max((len(c) for c in cols), default=1)
    W = ((max(wmax, 1) + 7) // 8) * 8

    xp = np.full((B, C_PAD), NEG, dtype=DT)
    xp[:, :C] = x

    z = np.zeros((B, 3, L, W), dtype=DT)
    z[:, 0, :, :] = NEG
    for l, cl in enumerate(cols):
        n = len(cl)
        if n:
            z[:, 0, l, :n] = x[:, cl]
            z[:, 1, l, :n] = y[:, cl]
            z[:, 2, l, :n] = y_neg[:, cl]
    z = z.reshape(B, 3 * L * W)

    key = (W, USE_BF16, SPLIT_X, ACT_Y)
    if key not in _graph_cache:
        _graph_cache[key] = _build(W)
    nc = _graph_cache[key]

    in_maps = [
        {"x": xp[i * ROWS:(i + 1) * ROWS], "z": z[i * ROWS:(i + 1) * ROWS]}
        for i in range(N_CORES)
    ]
    try:
        res = run_bass_kernel_spmd(nc, in_maps, core_ids=list(range(N_CORES)))
    except Exception:
        _reset_device()
        res = run_bass_kernel_spmd(nc, in_maps, core_ids=list(range(N_CORES)))
    LAST_RESULT = res

    loss = np.concatenate([res.results[i]["loss"].reshape(-1) for i in range(N_CORES)])
    return np.asarray(loss.mean(), dtype=np.float32)



# revision 2
# speedup vs baseline: 1.2164x; 1.2164x over previous
"""Trainium2 Bass kernel for nn_AsymmetricLossCustomMS.

Reference math per sample b (x, y, y_neg: [B, C]; group_mask: [L, C]):
  xs     = sigmoid(x)
  thres  = max(16th-largest of xs, 0.3)
  gmax_l = max over classes in group l of xs        (L groups)
  gt_l   = any positive y in group l; gt_neg_l likewise for y_neg
  caseB  = sum_l rank_loss picked by gt_l           (if any gt_l)
  caseA  = mix of union-max and neg-score rank losses (otherwise)
  loss   = mean over b

Strategy: pure data parallel over the batch (256 rows/core on 8 cores).
sigmoid is monotonic, so the 16th-largest and the group maxima are taken on
raw x and sigmoided afterwards (tiny [128, L] tensors).

Layout trick: the host permutes x columns so the whitelist classes come
first, each group padded with -1e30 to a fixed W-wide segment.  The group
maxima are then in-place slices of the same x row-tile (no second copy of
the whitelist values over HBM), and a column permutation doesn't change the
row top-k.

16th-largest per row: pairwise tensor_tensor-max folds (DVE runs those at 2
elem/cycle for bf16, vs 1 for MAX8) shrink the 9728-wide row to 152
candidates, then MAX8 -> MATCH_REPLACE8 -> MAX8 gives the 16th-largest of
the folded array exactly.  Folding to 152 slots loses a top-16 member only
when two of them collide in one slot (E[collisions] ~ 0.8/row, and losing
one just promotes the 17th-largest -- error ~1e-3 in sigmoid space).

y/y_neg: only whitelist columns matter; the host bit-packs them
(np.packbits) into 33 bytes per group segment, and one DVE max-reduce +
is_gt recovers the per-group any-positive flags for both row-tiles at once.
"""

import numpy as np

B, C, L = 2048, 9605, 8
N_CORES = 8
ROWS = B // N_CORES  # 256 rows per core
P = 128              # SBUF partitions per row-tile
TILES = ROWS // P    # 2 row-tiles per core
NEG = -1e30
ALPHA1 = 0.05  # margin
ALPHA3 = 5.0   # logistic sharpness
ALPHA_OTHER = 0.3
BIAS = ALPHA3 * ALPHA1

NCHUNK = 4     # x DMA chunks per row-tile

LAST_RESULT = None  # BassKernelResults of the most recent run (for test harness)

_graph_cache = {}


def _build(W, C_PAD):
    import concourse.bacc as bacc
    import concourse.tile as tile
    from concourse import mybir
    from concourse.alu_op_type import AluOpType as Op

    BF16 = mybir.dt.bfloat16
    F32 = mybir.dt.float32
    U8 = mybir.dt.uint8
    SIG = mybir.ActivationFunctionType.Sigmoid
    X = mybir.AxisListType.X

    S = C_PAD // 16      # fold-tree leaf width
    CW = 4 * S           # DMA chunk width
    WB = W // 8          # y bit-bytes per segment
    NWL = L * W          # whitelist block width

    nc = bacc.Bacc("TRN2", target_bir_lowering=False, debug=False, num_devices=N_CORES)
    x_d = nc.dram_tensor("x", [ROWS, C_PAD], BF16, kind="ExternalInput")
    zy_d = nc.dram_tensor("zy", [ROWS, 2 * L * WB], U8, kind="ExternalInput")
    out_d = nc.dram_tensor("loss", [P, TILES], F32, kind="ExternalOutput")

    with tile.TileContext(nc) as tc:
        with tc.tile_pool(name="consts", bufs=1) as consts, \
             tc.tile_pool(name="xbuf", bufs=1) as xbuf, \
             tc.tile_pool(name="scr", bufs=1) as scr, \
             tc.tile_pool(name="sm", bufs=1) as sm:
            bias_c = consts.tile([P, 1], F32)
            nc.gpsimd.memset(bias_c, BIAS)

            # --- input DMAs: x chunks from the sync engine, y bits from pool
            xt = [xbuf.tile([P, C_PAD], BF16, name=f"xt{t}") for t in range(TILES)]
            for t in range(TILES):
                for c in range(NCHUNK):
                    nc.sync.dma_start(
                        out=xt[t][:, c * CW:(c + 1) * CW],
                        in_=x_d.ap()[t * P:(t + 1) * P, c * CW:(c + 1) * CW],
                    )
            zy = sm.tile([P, TILES, 2 * L * WB], U8)
            nc.gpsimd.dma_start(
                out=zy, in_=zy_d.ap().rearrange("(t p) f -> p t f", t=TILES)
            )

            # --- shared small tensors (both tiles side by side in free dim)
            gm2 = sm.tile([P, TILES, L], F32)     # raw group maxima
            gsig2 = sm.tile([P, TILES, L], F32)   # sigmoid(group max)
            yr2 = sm.tile([P, TILES, 2 * L], F32)
            gt2 = sm.tile([P, TILES, 2 * L], F32)
            sgn2 = sm.tile([P, TILES, L], F32)
            negp2 = sm.tile([P, TILES, L], F32)
            un2 = sm.tile([P, TILES, 2], F32)
            dm2 = sm.tile([P, TILES, L], F32)
            dA2 = sm.tile([P, TILES, 2], F32)
            thr = [sm.tile([P, 1], F32, name=f"thr{t}") for t in range(TILES)]

            for t in range(TILES):
                xx = xt[t]
                # fold tree: per chunk 4*S -> S, then 4 -> 2 -> 1 -> S/2 -> S/4
                s1 = scr.tile([P, 8, S], BF16, name=f"s1_{t}")
                s2 = scr.tile([P, 4, S], BF16, name=f"s2_{t}")
                s3 = scr.tile([P, 2, S], BF16, name=f"s3_{t}")
                f1 = scr.tile([P, S], BF16, name=f"f1_{t}")
                f2 = scr.tile([P, S // 2], BF16, name=f"f2_{t}")
                f4 = scr.tile([P, S // 4], BF16, name=f"f4_{t}")
                for c in range(NCHUNK):
                    o = c * CW
                    nc.vector.tensor_tensor(
                        out=s1[:, 2 * c], in0=xx[:, o:o + S],
                        in1=xx[:, o + S:o + 2 * S], op=Op.max,
                    )
                    nc.vector.tensor_tensor(
                        out=s1[:, 2 * c + 1], in0=xx[:, o + 2 * S:o + 3 * S],
                        in1=xx[:, o + 3 * S:o + 4 * S], op=Op.max,
                    )
                    nc.vector.tensor_tensor(
                        out=s2[:, c], in0=s1[:, 2 * c], in1=s1[:, 2 * c + 1],
                        op=Op.max,
                    )
                    if c == 1:
                        # DVE is ahead of the DMA stream here: do the group-max
                        # folds (whitelist block lives inside chunk 0) and, for
                        # tile 0, the y-bit reduction while waiting for chunk 2.
                        v = xx[:, 0:NWL].rearrange("p (g w) -> p g w", w=W)
                        z1 = scr.tile([P, L, W // 2], BF16, name=f"z1_{t}")
                        z2 = scr.tile([P, L, W // 4], BF16, name=f"z2_{t}")
                        z3 = scr.tile([P, L, W // 8], BF16, name=f"z3_{t}")
                        nc.vector.tensor_tensor(
                            out=z1, in0=v[:, :, 0:W // 2],
                            in1=v[:, :, W // 2:W], op=Op.max,
                        )
                        nc.vector.tensor_tensor(
                            out=z2, in0=z1[:, :, 0:W // 4],
                            in1=z1[:, :, W // 4:W // 2], op=Op.max,
                        )
                        nc.vector.tensor_tensor(
                            out=z3, in0=z2[:, :, 0:W // 8],
                            in1=z2[:, :, W // 8:W // 4], op=Op.max,
                        )
                        nc.vector.tensor_reduce(
                            out=gm2[:, t], in_=z3, op=Op.max, axis=X
                        )
                        nc.scalar.activation(out=gsig2[:, t], in_=gm2[:, t], func=SIG)
                        if t == 0:
                            yv = zy.rearrange("p t (g w) -> p t g w", w=WB)
                            nc.vector.tensor_reduce(
                                out=yr2, in_=yv, op=Op.max, axis=X
                            )
                            nc.vector.tensor_scalar(
                                out=gt2, in0=yr2, scalar1=0.0, scalar2=None,
                                op0=Op.is_gt,
                            )
                            nc.gpsimd.tensor_scalar(
                                out=sgn2, in0=gt2[:, :, 0:L], scalar1=-2.0,
                                scalar2=1.0, op0=Op.mult, op1=Op.add,
                            )
                        # union max and neg score for this tile
                        nc.vector.reduce_max(
                            out=un2[:, t, 0:1], in_=gsig2[:, t], axis=X
                        )
                        nc.gpsimd.tensor_tensor(
                            out=negp2[:, t], in0=gt2[:, t, L:2 * L],
                            in1=gsig2[:, t], op=Op.mult,
                        )
                        nc.vector.reduce_max(
                            out=un2[:, t, 1:2], in_=negp2[:, t], axis=X
                        )
                nc.vector.tensor_tensor(
                    out=s3[:, 0], in0=s2[:, 0], in1=s2[:, 1], op=Op.max
                )
                nc.vector.tensor_tensor(
                    out=s3[:, 1], in0=s2[:, 2], in1=s2[:, 3], op=Op.max
                )
                nc.vector.tensor_tensor(
                    out=f1, in0=s3[:, 0], in1=s3[:, 1], op=Op.max
                )
                nc.vector.tensor_tensor(
                    out=f2, in0=f1[:, 0:S // 2], in1=f1[:, S // 2:S], op=Op.max
                )
                nc.vector.tensor_tensor(
                    out=f4, in0=f2[:, 0:S // 4], in1=f2[:, S // 4:S // 2], op=Op.max
                )
                # exact 16th-largest of the folded row
                g8 = sm.tile([P, 8], BF16, name=f"g8_{t}")
                nc.vector.max(out=g8, in_=f4)
                nc.vector.match_replace(
                    out=f4, in_to_replace=g8, in_values=f4, imm_value=NEG
                )
                n8 = sm.tile([P, 8], BF16, name=f"n8_{t}")
                nc.vector.max(out=n8, in_=f4)
                nc.scalar.activation(out=thr[t], in_=n8[:, 7:8], func=SIG)
                nc.vector.tensor_scalar_max(thr[t], thr[t], ALPHA_OTHER)

                # dm = (gsig - thres) * (1 - 2*gt);  dA = [umax, negscore] - thres
                nc.vector.scalar_tensor_tensor(
                    out=dm2[:, t], in0=gsig2[:, t], scalar=thr[t],
                    in1=sgn2[:, t], op0=Op.subtract, op1=Op.mult,
                )
                nc.vector.tensor_scalar(
                    out=dA2[:, t], in0=un2[:, t], scalar1=thr[t], scalar2=None,
                    op0=Op.subtract,
                )

            # --- combined tail over both tiles ---
            # caseB terms: sigmoid(5*dm + 0.25) * (1 + (dm > -0.05)), sum over l
            sB2 = sm.tile([P, TILES, L], F32)
            nc.scalar.activation(
                out=sB2, in_=dm2, func=SIG, scale=ALPHA3, bias=bias_c[:]
            )
            pB2 = sm.tile([P, TILES, L], F32)
            nc.gpsimd.tensor_scalar(
                out=pB2, in0=dm2, scalar1=-ALPHA1, scalar2=1.0,
                op0=Op.is_gt, op1=Op.add,
            )
            fB2 = sm.tile([P, TILES, L], F32)
            nc.vector.tensor_mul(fB2, sB2, pB2)
            caseB2 = sm.tile([P, TILES], F32)
            nc.vector.reduce_sum(out=caseB2, in_=fB2, axis=X)

            sA2 = sm.tile([P, TILES, 2], F32)
            nc.scalar.activation(
                out=sA2, in_=dA2, func=SIG, scale=ALPHA3, bias=bias_c[:]
            )
            pA2 = sm.tile([P, TILES, 2], F32)
            nc.gpsimd.tensor_scalar(
                out=pA2, in0=dA2, scalar1=-ALPHA1, scalar2=1.0,
                op0=Op.is_gt, op1=Op.add,
            )
            fA2 = sm.tile([P, TILES, 2], F32)
            nc.vector.tensor_mul(fA2, sA2, pA2)
            caseA2 = sm.tile([P, TILES], F32)
            nc.vector.reduce_sum(out=caseA2, in_=fA2, axis=X)
            nc.vector.tensor_scalar(
                out=caseA2, in0=caseA2, scalar1=0.5, scalar2=None, op0=Op.mult
            )

            # loss = caseA + has_gt * (caseB - caseA)
            hg2 = sm.tile([P, TILES], F32)
            nc.vector.reduce_max(out=hg2, in_=gt2[:, :, 0:L], axis=X)
            dd2 = sm.tile([P, TILES], F32)
            nc.vector.tensor_sub(dd2, caseB2, caseA2)
            nc.vector.tensor_mul(dd2, dd2, hg2)
            lossr2 = sm.tile([P, TILES], F32)
            nc.vector.tensor_add(lossr2, caseA2, dd2)
            nc.gpsimd.dma_start(out=out_d.ap(), in_=lossr2)
    nc.compile()
    return nc


def _reset_device():
    """Best-effort recovery of a wedged axon-tunneled NeuronCore."""
    import ctypes
    import time

    try:
        import jax

        jax.devices()
        lib = ctypes.CDLL("/opt/axon/libaxon_pjrt.so")
        lib.axon_reset.restype = ctypes.c_int64
        lib.axon_reset()
        time.sleep(45)
    except Exception:
        pass


def kernel(x, y, y_neg, group_mask):
    global LAST_RESULT
    from concourse.bass_utils import run_bass_kernel_spmd
    import ml_dtypes

    x = np.asarray(x, dtype=np.float32)
    y = np.asarray(y, dtype=np.float32)
    y_neg = np.asarray(y_neg, dtype=np.float32)
    gm = np.asarray(group_mask).astype(bool)
    BF16 = ml_dtypes.bfloat16

    cols = [np.flatnonzero(gm[l]) for l in range(L)]
    wmax = max((len(c) for c in cols), default=1)
    W = ((max(wmax, 1) + 7) // 8) * 8
    WB = W // 8
    NWL = L * W
    wl = np.concatenate(cols) if cols else np.zeros(0, np.int64)
    other = np.flatnonzero(~gm.any(axis=0))
    n_cols = NWL + len(other)
    C_PAD = ((n_cols + 15) // 16) * 16
    dst_wl = np.concatenate(
        [l * W + np.arange(len(cl)) for l, cl in enumerate(cols)]
    )

    # x with whitelist groups gathered to the front, NEG-padded segments
    xp = np.full((B, C_PAD), NEG, dtype=BF16)
    xb = x.astype(BF16)
    xp[:, dst_wl] = xb[:, wl]
    xp[:, NWL:NWL + len(other)] = xb[:, other]

    # bit-packed y / y_neg whitelist columns: [B, 2L, WB] uint8
    ybits = np.zeros((B, 2 * NWL), dtype=bool)
    ybits[:, dst_wl] = y[:, wl] != 0
    ybits[:, NWL + dst_wl] = y_neg[:, wl] != 0
    zyb = np.packbits(ybits.reshape(B, 2 * L, W), axis=-1).reshape(B, 2 * L * WB)

    key = (W, C_PAD)
    if key not in _graph_cache:
        _graph_cache[key] = _build(W, C_PAD)
    nc = _graph_cache[key]

    in_maps = [
        {"x": xp[i * ROWS:(i + 1) * ROWS], "zy": zyb[i * ROWS:(i + 1) * ROWS]}
        for i in range(N_CORES)
    ]
    try:
        res = run_bass_kernel_spmd(nc, in_maps, core_ids=list(range(N_CORES)))
    except Exception:
        _reset_device()
        res = run_bass_kernel_spmd(nc, in_maps, core_ids=list(range(N_CORES)))
    LAST_RESULT = res

    loss = np.concatenate([res.results[i]["loss"].reshape(-1) for i in range(N_CORES)])
    return np.asarray(loss.mean(), dtype=np.float32)


# revision 6
# speedup vs baseline: 1.2330x; 1.0136x over previous
"""Trainium2 Bass kernel for nn_AsymmetricLossCustomMS.

Reference math per sample b (x, y, y_neg: [B, C]; group_mask: [L, C]):
  xs     = sigmoid(x)
  thres  = max(16th-largest of xs, 0.3)
  gmax_l = max over classes in group l of xs        (L groups)
  gt_l   = any positive y in group l; gt_neg_l likewise for y_neg
  caseB  = sum_l rank_loss picked by gt_l           (if any gt_l)
  caseA  = mix of union-max and neg-score rank losses (otherwise)
  loss   = mean over b

Strategy: pure data parallel over the batch (256 rows/core on 8 cores).
sigmoid is monotonic, so the 16th-largest and the group maxima are taken on
raw x and sigmoided afterwards (tiny [128, L] tensors).

Layout trick: the host permutes x columns so the whitelist classes come
first, each group padded with -1e30 to a fixed W-wide segment.  The group
maxima are then in-place slices of the same x row-tile (no second copy of
the whitelist values over HBM), and a column permutation doesn't change the
row top-k.

16th-largest per row: pairwise tensor_tensor-max folds (DVE runs those at 2
elem/cycle for bf16, vs 1 for MAX8) shrink the 9728-wide row to 152
candidates, then MAX8 -> MATCH_REPLACE8 -> MAX8 gives the 16th-largest of
the folded array exactly.  Folding to 152 slots loses a top-16 member only
when two of them collide in one slot (E[collisions] ~ 0.8/row, and losing
one just promotes the 17th-largest -- error ~1e-3 in sigmoid space).

y/y_neg: only whitelist columns matter; the host bit-packs them
(np.packbits) into 33 bytes per group segment, and one DVE max-reduce +
is_gt recovers the per-group any-positive flags for both row-tiles at once.
"""

import numpy as np

B, C, L = 2048, 9605, 8
N_CORES = 8
ROWS = B // N_CORES  # 256 rows per core
P = 128              # SBUF partitions per row-tile
TILES = ROWS // P    # 2 row-tiles per core
NEG = -1e30
ALPHA1 = 0.05  # margin
ALPHA3 = 5.0   # logistic sharpness
ALPHA_OTHER = 0.3
BIAS = ALPHA3 * ALPHA1

NCHUNK = 4     # x DMA chunks per row-tile

LAST_RESULT = None  # BassKernelResults of the most recent run (for test harness)

_graph_cache = {}


def _build(W, C_PAD):
    import concourse.bacc as bacc
    import concourse.tile as tile
    from concourse import mybir
    from concourse.alu_op_type import AluOpType as Op

    BF16 = mybir.dt.bfloat16
    F32 = mybir.dt.float32
    U8 = mybir.dt.uint8
    SIG = mybir.ActivationFunctionType.Sigmoid
    X = mybir.AxisListType.X

    S = C_PAD // 16      # fold-tree leaf width
    WB = W // 8          # y bit-bytes per segment
    NWL = L * W          # whitelist block width
    # x DMA chunks per tile, in S units: three 4S chunks then two small 2S
    # ones so the post-stream fold tail after the last arrival is short.
    CHUNKS = [4, 4, 4, 2, 2]
    BOUNDS = [0]
    for k in CHUNKS:
        BOUNDS.append(BOUNDS[-1] + k * S)

    nc = bacc.Bacc("TRN2", target_bir_lowering=False, debug=False, num_devices=N_CORES)
    x_d = nc.dram_tensor("x", [ROWS, C_PAD], BF16, kind="ExternalInput")
    zy_d = nc.dram_tensor("zy", [ROWS, 2 * L * WB], U8, kind="ExternalInput")
    out_d = nc.dram_tensor("loss", [P, TILES], F32, kind="ExternalOutput")

    with tile.TileContext(nc) as tc:
        with tc.tile_pool(name="consts", bufs=1) as consts, \
             tc.tile_pool(name="xbuf", bufs=1) as xbuf, \
             tc.tile_pool(name="scr", bufs=1) as scr, \
             tc.tile_pool(name="sm", bufs=1) as sm:
            xt = [xbuf.tile([P, C_PAD], BF16, name=f"xt{t}") for t in range(TILES)]

            # --- input DMAs.  The DMA fabric serves queues at aggregate
            # bandwidth in rough issue order, so: first x chunk from gpsimd
            # (its preamble finishes first), the rest from the idle sync
            # engine, and the y bits last (their consumers have slack).
            nc.gpsimd.dma_start(
                out=xt[0][:, BOUNDS[0]:BOUNDS[1]],
                in_=x_d.ap()[0:P, BOUNDS[0]:BOUNDS[1]],
            )
            bias_c = consts.tile([P, 1], F32)
            nc.gpsimd.memset(bias_c, BIAS)
            for t in range(TILES):
                for c in range(len(CHUNKS)):
                    if t == 0 and c == 0:
                        continue
                    nc.sync.dma_start(
                        out=xt[t][:, BOUNDS[c]:BOUNDS[c + 1]],
                        in_=x_d.ap()[t * P:(t + 1) * P, BOUNDS[c]:BOUNDS[c + 1]],
                    )
            zy = sm.tile([P, TILES, 2 * L * WB], U8)
            nc.sync.dma_start(
                out=zy, in_=zy_d.ap().rearrange("(t p) f -> p t f", t=TILES)
            )

            # --- shared small tensors (both tiles side by side in free dim)
            gm2 = sm.tile([P, TILES, L], F32)     # raw group maxima
            gsig2 = sm.tile([P, TILES, L], F32)   # sigmoid(group max)
            yr2 = sm.tile([P, TILES, 2 * L], F32)
            gt2 = sm.tile([P, TILES, 2 * L], F32)
            sgn2 = sm.tile([P, TILES, L], F32)
            negp2 = sm.tile([P, TILES, L], F32)
            un2 = sm.tile([P, TILES, 2], F32)
            dm2 = sm.tile([P, TILES, L], F32)
            dA2 = sm.tile([P, TILES, 2], F32)
            thr = [sm.tile([P, 1], F32, name=f"thr{t}") for t in range(TILES)]

            for t in range(TILES):
                xx = xt[t]
                # per-chunk fold to S candidates, merged into a running acc
                s1 = scr.tile([P, 2, S], BF16, name=f"s1_{t}")
                tmp = scr.tile([P, S], BF16, name=f"tmp_{t}")
                acc = scr.tile([P, S], BF16, name=f"acc_{t}")
                f2 = scr.tile([P, S // 2], BF16, name=f"f2_{t}")
                f4 = scr.tile([P, S // 4], BF16, name=f"f4_{t}")
                for c, k in enumerate(CHUNKS):
                    o = BOUNDS[c]
                    dst = acc if c == 0 else tmp
                    if k == 4:
                        nc.vector.tensor_tensor(
                            out=s1[:, 0], in0=xx[:, o:o + S],
                            in1=xx[:, o + S:o + 2 * S], op=Op.max,
                        )
                        nc.vector.tensor_tensor(
                            out=s1[:, 1], in0=xx[:, o + 2 * S:o + 3 * S],
                            in1=xx[:, o + 3 * S:o + 4 * S], op=Op.max,
                        )
                        nc.vector.tensor_tensor(
                            out=dst, in0=s1[:, 0], in1=s1[:, 1], op=Op.max
                        )
                    else:
                        nc.vector.tensor_tensor(
                            out=dst, in0=xx[:, o:o + S],
                            in1=xx[:, o + S:o + 2 * S], op=Op.max,
                        )
                    if c > 0:
                        nc.vector.tensor_tensor(
                            out=acc, in0=acc, in1=tmp, op=Op.max
                        )
                    if c == 1:
                        # DVE is ahead of the DMA stream here: do the group-max
                        # folds (whitelist block lives inside chunk 0) and, for
                        # tile 0, the y-bit reduction while waiting for chunk 2.
                        v = xx[:, 0:NWL].rearrange("p (g w) -> p g w", w=W)
                        z1 = scr.tile([P, L, W // 2], BF16, name=f"z1_{t}")
                        z2 = scr.tile([P, L, W // 4], BF16, name=f"z2_{t}")
                        z3 = scr.tile([P, L, W // 8], BF16, name=f"z3_{t}")
                        nc.vector.tensor_tensor(
                            out=z1, in0=v[:, :, 0:W // 2],
                            in1=v[:, :, W // 2:W], op=Op.max,
                        )
                        nc.vector.tensor_tensor(
                            out=z2, in0=z1[:, :, 0:W // 4],
                            in1=z1[:, :, W // 4:W // 2], op=Op.max,
                        )
                        nc.vector.tensor_tensor(
                            out=z3, in0=z2[:, :, 0:W // 8],
                            in1=z2[:, :, W // 8:W // 4], op=Op.max,
                        )
                        nc.vector.tensor_reduce(
                            out=gm2[:, t], in_=z3, op=Op.max, axis=X
                        )
                        nc.scalar.activation(out=gsig2[:, t], in_=gm2[:, t], func=SIG)
                        if t == 0:
                            yv = zy.rearrange("p t (g w) -> p t g w", w=WB)
                            nc.vector.tensor_reduce(
                                out=yr2, in_=yv, op=Op.max, axis=X
                            )
                            nc.vector.tensor_scalar(
                                out=gt2, in0=yr2, scalar1=0.0, scalar2=None,
                                op0=Op.is_gt,
                            )
                            nc.gpsimd.tensor_scalar(
                                out=sgn2, in0=gt2[:, :, 0:L], scalar1=-2.0,
                                scalar2=1.0, op0=Op.mult, op1=Op.add,
                            )
                        # union max and neg score for this tile
                        nc.vector.reduce_max(
                            out=un2[:, t, 0:1], in_=gsig2[:, t], axis=X
                        )
                        nc.gpsimd.tensor_tensor(
                            out=negp2[:, t], in0=gt2[:, t, L:2 * L],
                            in1=gsig2[:, t], op=Op.mult,
                        )
                        nc.vector.reduce_max(
                            out=un2[:, t, 1:2], in_=negp2[:, t], axis=X
                        )
                nc.vector.tensor_tensor(
                    out=f2, in0=acc[:, 0:S // 2], in1=acc[:, S // 2:S], op=Op.max
                )
                nc.vector.tensor_tensor(
                    out=f4, in0=f2[:, 0:S // 4], in1=f2[:, S // 4:S // 2], op=Op.max
                )
                # exact 16th-largest of the folded row
                g8 = sm.tile([P, 8], BF16, name=f"g8_{t}")
                nc.vector.max(out=g8, in_=f4)
                nc.vector.match_replace(
                    out=f4, in_to_replace=g8, in_values=f4, imm_value=NEG
                )
                n8 = sm.tile([P, 8], BF16, name=f"n8_{t}")
                nc.vector.max(out=n8, in_=f4)
                nc.scalar.activation(out=thr[t], in_=n8[:, 7:8], func=SIG)
                nc.vector.tensor_scalar_max(thr[t], thr[t], ALPHA_OTHER)

                # dm = (gsig - thres) * (1 - 2*gt);  dA = [umax, negscore] - thres
                nc.vector.scalar_tensor_tensor(
                    out=dm2[:, t], in0=gsig2[:, t], scalar=thr[t],
                    in1=sgn2[:, t], op0=Op.subtract, op1=Op.mult,
                )
                nc.vector.tensor_scalar(
                    out=dA2[:, t], in0=un2[:, t], scalar1=thr[t], scalar2=None,
                    op0=Op.subtract,
                )

            # --- combined tail over both tiles ---
            # caseB terms: sigmoid(5*dm + 0.25) * (1 + (dm > -0.05)), sum over l
            sB2 = sm.tile([P, TILES, L], F32)
            nc.scalar.activation(
                out=sB2, in_=dm2, func=SIG, scale=ALPHA3, bias=bias_c[:]
            )
            pB2 = sm.tile([P, TILES, L], F32)
            nc.gpsimd.tensor_scalar(
                out=pB2, in0=dm2, scalar1=-ALPHA1, scalar2=1.0,
                op0=Op.is_gt, op1=Op.add,
            )
            fB2 = sm.tile([P, TILES, L], F32)
            nc.vector.tensor_mul(fB2, sB2, pB2)
            caseB2 = sm.tile([P, TILES], F32)
            nc.vector.reduce_sum(out=caseB2, in_=fB2, axis=X)

            sA2 = sm.tile([P, TILES, 2], F32)
            nc.scalar.activation(
                out=sA2, in_=dA2, func=SIG, scale=ALPHA3, bias=bias_c[:]
            )
            pA2 = sm.tile([P, TILES, 2], F32)
            nc.gpsimd.tensor_scalar(
                out=pA2, in0=dA2, scalar1=-ALPHA1, scalar2=1.0,
                op0=Op.is_gt, op1=Op.add,
            )
            fA2 = sm.tile([P, TILES, 2], F32)
            nc.vector.tensor_mul(fA2, sA2, pA2)
            caseAr2 = sm.tile([P, TILES], F32)
            nc.vector.reduce_sum(out=caseAr2, in_=fA2, axis=X)
            caseA2 = sm.tile([P, TILES], F32)
            nc.gpsimd.tensor_scalar(
                out=caseA2, in0=caseAr2, scalar1=0.5, scalar2=None, op0=Op.mult
            )

            # loss = caseA + has_gt * (caseB - caseA);  caseA = 0.5 * caseAr
            hg2 = sm.tile([P, TILES], F32)
            nc.vector.reduce_max(out=hg2, in_=gt2[:, :, 0:L], axis=X)
            dd2 = sm.tile([P, TILES], F32)
            nc.vector.scalar_tensor_tensor(
                out=dd2, in0=caseAr2, scalar=-0.5, in1=caseB2,
                op0=Op.mult, op1=Op.add,
            )
            nc.vector.tensor_mul(dd2, dd2, hg2)
            lossr2 = sm.tile([P, TILES], F32)
            nc.vector.tensor_add(lossr2, caseA2, dd2)
            nc.gpsimd.dma_start(out=out_d.ap(), in_=lossr2)
    nc.compile()
    return nc


def _reset_device():
    """Best-effort recovery of a wedged axon-tunneled NeuronCore."""
    import ctypes
    import time

    try:
        import jax

        jax.devices()
        lib = ctypes.CDLL("/opt/axon/libaxon_pjrt.so")
        lib.axon_reset.restype = ctypes.c_int64
        lib.axon_reset()
        time.sleep(45)
    except Exception:
        pass


def kernel(x, y, y_neg, group_mask):
    global LAST_RESULT
    from concourse.bass_utils import run_bass_kernel_spmd
    import ml_dtypes

    x = np.asarray(x, dtype=np.float32)
    y = np.asarray(y, dtype=np.float32)
    y_neg = np.asarray(y_neg, dtype=np.float32)
    gm = np.asarray(group_mask).astype(bool)
    BF16 = ml_dtypes.bfloat16

    cols = [np.flatnonzero(gm[l]) for l in range(L)]
    wmax = max((len(c) for c in cols), default=1)
    W = ((max(wmax, 1) + 7) // 8) * 8
    WB = W // 8
    NWL = L * W
    wl = np.concatenate(cols) if cols else np.zeros(0, np.int64)
    other = np.flatnonzero(~gm.any(axis=0))
    n_cols = NWL + len(other)
    C_PAD = ((n_cols + 15) // 16) * 16
    dst_wl = np.concatenate(
        [l * W + np.arange(len(cl)) for l, cl in enumerate(cols)]
    )

    # x with whitelist groups gathered to the front, NEG-padded segments
    xp = np.full((B, C_PAD), NEG, dtype=BF16)
    xb = x.astype(BF16)
    xp[:, dst_wl] = xb[:, wl]
    xp[:, NWL:NWL + len(other)] = xb[:, other]

    # bit-packed y / y_neg whitelist columns: [B, 2L, WB] uint8
    ybits = np.zeros((B, 2 * NWL), dtype=bool)
    ybits[:, dst_wl] = y[:, wl] != 0
    ybits[:, NWL + dst_wl] = y_neg[:, wl] != 0
    zyb = np.packbits(ybits.reshape(B, 2 * L, W), axis=-1).reshape(B, 2 * L * WB)

    key = (W, C_PAD)
    if key not in _graph_cache:
        _graph_cache[key] = _build(W, C_PAD)
    nc = _graph_cache[key]

    in_maps = [
        {"x": xp[i * ROWS:(i + 1) * ROWS], "zy": zyb[i * ROWS:(i + 1) * ROWS]}
        for i in range(N_CORES)
    ]
    try:
        res = run_bass_kernel_spmd(nc, in_maps, core_ids=list(range(N_CORES)))
    except Exception:
        _reset_device()
        res = run_bass_kernel_spmd(nc, in_maps, core_ids=list(range(N_CORES)))
    LAST_RESULT = res

    loss = np.concatenate([res.results[i]["loss"].reshape(-1) for i in range(N_CORES)])
    return np.asarray(loss.mean(), dtype=np.float32)


# revision 9
# speedup vs baseline: 1.3433x; 1.0895x over previous
"""Trainium2 Bass kernel for nn_AsymmetricLossCustomMS.

Reference math per sample b (x, y, y_neg: [B, C]; group_mask: [L, C]):
  xs     = sigmoid(x)
  thres  = max(16th-largest of xs, 0.3)
  gmax_l = max over classes in group l of xs        (L groups)
  gt_l   = any positive y in group l; gt_neg_l likewise for y_neg
  caseB  = sum_l rank_loss picked by gt_l           (if any gt_l)
  caseA  = mix of union-max and neg-score rank losses (otherwise)
  loss   = mean over b

Strategy: pure data parallel over the batch (256 rows/core on 8 cores).
sigmoid is monotonic, so the 16th-largest and the group maxima are taken on
raw x and sigmoided afterwards (tiny [128, L] tensors).

Layout trick: the host permutes x columns so the whitelist classes come
first, each group padded with -1e30 to a fixed W-wide segment.  The group
maxima are then in-place slices of the same x row-tile (no second copy of
the whitelist values over HBM), and a column permutation doesn't change the
row top-k.

16th-largest per row: pairwise tensor_tensor-max folds (DVE runs those at 2
elem/cycle for bf16, vs 1 for MAX8) shrink the 9728-wide row to 152
candidates, then MAX8 -> MATCH_REPLACE8 -> MAX8 gives the 16th-largest of
the folded array exactly.  Folding to 152 slots loses a top-16 member only
when two of them collide in one slot (E[collisions] ~ 0.8/row, and losing
one just promotes the 17th-largest -- error ~1e-3 in sigmoid space).

y/y_neg: only whitelist columns matter; the host bit-packs them
(np.packbits) into 33 bytes per group segment, and one DVE max-reduce +
is_gt recovers the per-group any-positive flags for both row-tiles at once.
"""

import numpy as np

B, C, L = 2048, 9605, 8
N_CORES = 8
ROWS = B // N_CORES  # 256 rows per core
P = 128              # SBUF partitions per row-tile
TILES = ROWS // P    # 2 row-tiles per core
NEG = -1e30
ALPHA1 = 0.05  # margin
ALPHA3 = 5.0   # logistic sharpness
ALPHA_OTHER = 0.3
BIAS = ALPHA3 * ALPHA1

NCHUNK = 4     # x DMA chunks per row-tile

LAST_RESULT = None  # BassKernelResults of the most recent run (for test harness)

_graph_cache = {}


def _build(W, C_PAD):
    import concourse.bacc as bacc
    import concourse.tile as tile
    from concourse import mybir
    from concourse.alu_op_type import AluOpType as Op

    BF16 = mybir.dt.bfloat16
    F32 = mybir.dt.float32
    U8 = mybir.dt.uint8
    SIG = mybir.ActivationFunctionType.Sigmoid
    X = mybir.AxisListType.X

    S = C_PAD // 16      # fold-tree leaf width
    WB = W // 8          # y bit-bytes per segment
    NWL = L * W          # whitelist block width
    # x DMA chunks per tile, in S units: small leading chunks so the first
    # fold starts early, small trailing ones so the post-stream fold tail
    # after the last arrival is short.
    CHUNKS = [2, 2, 4, 4, 2, 2]
    BOUNDS = [0]
    for k in CHUNKS:
        BOUNDS.append(BOUNDS[-1] + k * S)

    nc = bacc.Bacc("TRN2", target_bir_lowering=False, debug=False, num_devices=N_CORES)
    x_d = nc.dram_tensor("x", [ROWS, C_PAD], BF16, kind="ExternalInput")
    zy_d = nc.dram_tensor("zy", [ROWS, 2 * L * WB], U8, kind="ExternalInput")
    out_d = nc.dram_tensor("loss", [P, TILES], F32, kind="ExternalOutput")

    with tile.TileContext(nc) as tc:
        with tc.tile_pool(name="consts", bufs=1) as consts, \
             tc.tile_pool(name="xbuf", bufs=1) as xbuf, \
             tc.tile_pool(name="scr", bufs=1) as scr, \
             tc.tile_pool(name="sm", bufs=1) as sm:
            xt = [xbuf.tile([P, C_PAD], BF16, name=f"xt{t}") for t in range(TILES)]

            # --- input DMAs, all from the sync engine: its hardware-dynamic
            # DMA queue runs at full fabric bandwidth and serves transfers in
            # issue order (gpsimd's software queue dribbles at ~20 B/ns).
            # y bits go last -- their consumers have the most slack.
            bias_c = consts.tile([P, 1], F32)
            nc.gpsimd.memset(bias_c, BIAS)
            for t in range(TILES):
                for c in range(len(CHUNKS)):
                    nc.sync.dma_start(
                        out=xt[t][:, BOUNDS[c]:BOUNDS[c + 1]],
                        in_=x_d.ap()[t * P:(t + 1) * P, BOUNDS[c]:BOUNDS[c + 1]],
                    )
            zy = sm.tile([P, TILES, 2 * L * WB], U8)
            nc.sync.dma_start(
                out=zy, in_=zy_d.ap().rearrange("(t p) f -> p t f", t=TILES)
            )

            # --- shared small tensors (both tiles side by side in free dim)
            gm2 = sm.tile([P, TILES, L], F32)     # raw group maxima
            gsig2 = sm.tile([P, TILES, L], F32)   # sigmoid(group max)
            yr2 = sm.tile([P, TILES, 2 * L], F32)
            gt2 = sm.tile([P, TILES, 2 * L], F32)
            sgn2 = sm.tile([P, TILES, L], F32)
            negp2 = sm.tile([P, TILES, L], F32)
            un2 = sm.tile([P, TILES, 2], F32)
            dm2 = sm.tile([P, TILES, L], F32)
            dA2 = sm.tile([P, TILES, 2], F32)
            thr = [sm.tile([P, 1], F32, name=f"thr{t}") for t in range(TILES)]

            for t in range(TILES):
                xx = xt[t]
                # per-chunk fold to S candidates, merged into a running acc
                s1 = scr.tile([P, 2, S], BF16, name=f"s1_{t}")
                tmp = scr.tile([P, S], BF16, name=f"tmp_{t}")
                acc = scr.tile([P, S], BF16, name=f"acc_{t}")
                f2 = scr.tile([P, S // 2], BF16, name=f"f2_{t}")
                f4 = scr.tile([P, S // 4], BF16, name=f"f4_{t}")
                for c, k in enumerate(CHUNKS):
                    o = BOUNDS[c]
                    dst = acc if c == 0 else tmp
                    if k == 4:
                        nc.vector.tensor_tensor(
                            out=s1[:, 0], in0=xx[:, o:o + S],
                            in1=xx[:, o + S:o + 2 * S], op=Op.max,
                        )
                        nc.vector.tensor_tensor(
                            out=s1[:, 1], in0=xx[:, o + 2 * S:o + 3 * S],
                            in1=xx[:, o + 3 * S:o + 4 * S], op=Op.max,
                        )
                        nc.vector.tensor_tensor(
                            out=dst, in0=s1[:, 0], in1=s1[:, 1], op=Op.max
                        )
                    else:
                        nc.vector.tensor_tensor(
                            out=dst, in0=xx[:, o:o + S],
                            in1=xx[:, o + S:o + 2 * S], op=Op.max,
                        )
                    if c > 0:
                        nc.vector.tensor_tensor(
                            out=acc, in0=acc, in1=tmp, op=Op.max
                        )
                    if c == 1:
                        # DVE is ahead of the DMA stream here: do the group-max
                        # folds (whitelist block lives inside chunk 0) and, for
                        # tile 0, the y-bit reduction while waiting for chunk 2.
                        v = xx[:, 0:NWL].rearrange("p (g w) -> p g w", w=W)
                        z1 = scr.tile([P, L, W // 2], BF16, name=f"z1_{t}")
                        z2 = scr.tile([P, L, W // 4], BF16, name=f"z2_{t}")
                        z3 = scr.tile([P, L, W // 8], BF16, name=f"z3_{t}")
                        nc.vector.tensor_tensor(
                            out=z1, in0=v[:, :, 0:W // 2],
                            in1=v[:, :, W // 2:W], op=Op.max,
                        )
                        nc.vector.tensor_tensor(
                            out=z2, in0=z1[:, :, 0:W // 4],
                            in1=z1[:, :, W // 4:W // 2], op=Op.max,
                        )
                        nc.vector.tensor_tensor(
                            out=z3, in0=z2[:, :, 0:W // 8],
                            in1=z2[:, :, W // 8:W // 4], op=Op.max,
                        )
                        nc.vector.tensor_reduce(
                            out=gm2[:, t], in_=z3, op=Op.max, axis=X
                        )
                        nc.scalar.activation(out=gsig2[:, t], in_=gm2[:, t], func=SIG)
                        if t == 0:
                            yv = zy.rearrange("p t (g w) -> p t g w", w=WB)
                            nc.vector.tensor_reduce(
                                out=yr2, in_=yv, op=Op.max, axis=X
                            )
                            nc.vector.tensor_scalar(
                                out=gt2, in0=yr2, scalar1=0.0, scalar2=None,
                                op0=Op.is_gt,
                            )
                            nc.gpsimd.tensor_scalar(
                                out=sgn2, in0=gt2[:, :, 0:L], scalar1=-2.0,
                                scalar2=1.0, op0=Op.mult, op1=Op.add,
                            )
                        # union max and neg score for this tile
                        nc.vector.reduce_max(
                            out=un2[:, t, 0:1], in_=gsig2[:, t], axis=X
                        )
                        nc.gpsimd.tensor_tensor(
                            out=negp2[:, t], in0=gt2[:, t, L:2 * L],
                            in1=gsig2[:, t], op=Op.mult,
                        )
                        nc.vector.reduce_max(
                            out=un2[:, t, 1:2], in_=negp2[:, t], axis=X
                        )
                nc.vector.tensor_tensor(
                    out=f2, in0=acc[:, 0:S // 2], in1=acc[:, S // 2:S], op=Op.max
                )
                nc.vector.tensor_tensor(
                    out=f4, in0=f2[:, 0:S // 4], in1=f2[:, S // 4:S // 2], op=Op.max
                )
                # exact 16th-largest of the folded row
                g8 = sm.tile([P, 8], BF16, name=f"g8_{t}")
                nc.vector.max(out=g8, in_=f4)
                nc.vector.match_replace(
                    out=f4, in_to_replace=g8, in_values=f4, imm_value=NEG
                )
                n8 = sm.tile([P, 8], BF16, name=f"n8_{t}")
                nc.vector.max(out=n8, in_=f4)
                nc.scalar.activation(out=thr[t], in_=n8[:, 7:8], func=SIG)
                nc.vector.tensor_scalar_max(thr[t], thr[t], ALPHA_OTHER)

                # dm = (gsig - thres) * (1 - 2*gt);  dA = [umax, negscore] - thres
                nc.vector.scalar_tensor_tensor(
                    out=dm2[:, t], in0=gsig2[:, t], scalar=thr[t],
                    in1=sgn2[:, t], op0=Op.subtract, op1=Op.mult,
                )
                nc.vector.tensor_scalar(
                    out=dA2[:, t], in0=un2[:, t], scalar1=thr[t], scalar2=None,
                    op0=Op.subtract,
                )

            # --- combined tail over both tiles ---
            # caseB terms: sigmoid(5*dm + 0.25) * (1 + (dm > -0.05)), sum over l
            sB2 = sm.tile([P, TILES, L], F32)
            nc.scalar.activation(
                out=sB2, in_=dm2, func=SIG, scale=ALPHA3, bias=bias_c[:]
            )
            pB2 = sm.tile([P, TILES, L], F32)
            nc.gpsimd.tensor_scalar(
                out=pB2, in0=dm2, scalar1=-ALPHA1, scalar2=1.0,
                op0=Op.is_gt, op1=Op.add,
            )
            fB2 = sm.tile([P, TILES, L], F32)
            nc.vector.tensor_mul(fB2, sB2, pB2)
            caseB2 = sm.tile([P, TILES], F32)
            nc.vector.reduce_sum(out=caseB2, in_=fB2, axis=X)

            sA2 = sm.tile([P, TILES, 2], F32)
            nc.scalar.activation(
                out=sA2, in_=dA2, func=SIG, scale=ALPHA3, bias=bias_c[:]
            )
            pA2 = sm.tile([P, TILES, 2], F32)
            nc.gpsimd.tensor_scalar(
                out=pA2, in0=dA2, scalar1=-ALPHA1, scalar2=1.0,
                op0=Op.is_gt, op1=Op.add,
            )
            fA2 = sm.tile([P, TILES, 2], F32)
            nc.vector.tensor_mul(fA2, sA2, pA2)
            caseAr2 = sm.tile([P, TILES], F32)
            nc.vector.reduce_sum(out=caseAr2, in_=fA2, axis=X)
            caseA2 = sm.tile([P, TILES], F32)
            nc.gpsimd.tensor_scalar(
                out=caseA2, in0=caseAr2, scalar1=0.5, scalar2=None, op0=Op.mult
            )

            # loss = caseA + has_gt * (caseB - caseA);  caseA = 0.5 * caseAr
            hg2 = sm.tile([P, TILES], F32)
            nc.vector.reduce_max(out=hg2, in_=gt2[:, :, 0:L], axis=X)
            dd2 = sm.tile([P, TILES], F32)
            nc.vector.scalar_tensor_tensor(
                out=dd2, in0=caseAr2, scalar=-0.5, in1=caseB2,
                op0=Op.mult, op1=Op.add,
            )
            nc.vector.tensor_mul(dd2, dd2, hg2)
            lossr2 = sm.tile([P, TILES], F32)
            nc.vector.tensor_add(lossr2, caseA2, dd2)
            nc.sync.dma_start(out=out_d.ap(), in_=lossr2)
    nc.compile()
    return nc


def _reset_device():
    """Best-effort recovery of a wedged axon-tunneled NeuronCore."""
    import ctypes
    import time

    try:
        import jax

        jax.devices()
        lib = ctypes.CDLL("/opt/axon/libaxon_pjrt.so")
        lib.axon_reset.restype = ctypes.c_int64
        lib.axon_reset()
        time.sleep(45)
    except Exception:
        pass


def kernel(x, y, y_neg, group_mask):
    global LAST_RESULT
    from concourse.bass_utils import run_bass_kernel_spmd
    import ml_dtypes

    x = np.asarray(x, dtype=np.float32)
    y = np.asarray(y, dtype=np.float32)
    y_neg = np.asarray(y_neg, dtype=np.float32)
    gm = np.asarray(group_mask).astype(bool)
    BF16 = ml_dtypes.bfloat16

    cols = [np.flatnonzero(gm[l]) for l in range(L)]
    wmax = max((len(c) for c in cols), default=1)
    W = ((max(wmax, 1) + 7) // 8) * 8
    WB = W // 8
    NWL = L * W
    wl = np.concatenate(cols) if cols else np.zeros(0, np.int64)
    other = np.flatnonzero(~gm.any(axis=0))
    n_cols = NWL + len(other)
    C_PAD = ((n_cols + 15) // 16) * 16
    dst_wl = np.concatenate(
        [l * W + np.arange(len(cl)) for l, cl in enumerate(cols)]
    )

    # x with whitelist groups gathered to the front, NEG-padded segments
    xp = np.full((B, C_PAD), NEG, dtype=BF16)
    xb = x.astype(BF16)
    xp[:, dst_wl] = xb[:, wl]
    xp[:, NWL:NWL + len(other)] = xb[:, other]

    # bit-packed y / y_neg whitelist columns: [B, 2L, WB] uint8
    ybits = np.zeros((B, 2 * NWL), dtype=bool)
    ybits[:, dst_wl] = y[:, wl] != 0
    ybits[:, NWL + dst_wl] = y_neg[:, wl] != 0
    zyb = np.packbits(ybits.reshape(B, 2 * L, W), axis=-1).reshape(B, 2 * L * WB)

    key = (W, C_PAD)
    if key not in _graph_cache:
        _graph_cache[key] = _build(W, C_PAD)
    nc = _graph_cache[key]

    in_maps = [
        {"x": xp[i * ROWS:(i + 1) * ROWS], "zy": zyb[i * ROWS:(i + 1) * ROWS]}
        for i in range(N_CORES)
    ]
    try:
        res = run_bass_kernel_spmd(nc, in_maps, core_ids=list(range(N_CORES)))
    except Exception:
        _reset_device()
        res = run_bass_kernel_spmd(nc, in_maps, core_ids=list(range(N_CORES)))
    LAST_RESULT = res

    loss = np.concatenate([res.results[i]["loss"].reshape(-1) for i in range(N_CORES)])
    return np.asarray(loss.mean(), dtype=np.float32)
